# revision 25
# baseline (speedup 1.0000x reference)
"""GCN (PyG GCNConv + 3-layer MLP head) on 8 Trainium2 NeuronCores.

Strategy (graph/data parallel, hardcoded for N=100000, F=512, E=3200000):
  - Nodes sharded 12500/core (core c owns targets [12500c, 12500(c+1))).
  - Phase A (per core): stream x-shard (transposed, bf16) through TensorE
    to get ytil.T = dinv * (x @ Wg) for the local shard.
  - Phase B: AllGather ytil across the 8 cores; lay the full [100K, 16]
    ytil.T into SBUF as 4 windows x 2 replicas x 16 features = 128
    partitions (window = 25000 nodes along the free axis).
  - Phase C: per-edge aggregation with GPSIMD ap_gather: edges (incl.
    self-loops) are bucketed by (source window, replica) into the 8
    16-partition groups; per (target, group) slot lists are padded to a
    shared per-target cap m(t) so one VectorE tensor_reduce per degree
    class computes all segment sums with a uniform access pattern.
  - Phase D: fold the 8 groups' partials with a one-hot selector matmul,
    scale by dinv[target], + bias, ReLU, then the 16->16->32->1 MLP with
    fused bias/ReLU/sigmoid on ScalarE.  Targets are processed in a
    degree-class-sorted column order; the host inverts the permutation.

Self-contained: only needs /opt/trn_rl_repo (concourse/Bass) and the
axon-attached NeuronCores visible through jax.
"""
import sys
import types

if "/opt/trn_rl_repo" not in sys.path:
    sys.path.insert(0, "/opt/trn_rl_repo")

# bass_utils' trace path imports antenv.axon_hooks, which the image's antenv
# lacks; stub it so the import never fails (we run with trace=False here).
try:
    import antenv  # noqa: F401
    import antenv.axon_hooks  # noqa: F401
except Exception:
    _hooks = types.ModuleType("antenv.axon_hooks")
    _hooks._hook = None
    _hooks.set_axon_ntff_profile_hook = lambda h: setattr(_hooks, "_hook", h)
    _hooks.get_axon_ntff_profile_hook = lambda: _hooks._hook
    sys.modules["antenv.axon_hooks"] = _hooks

import numpy as np

import concourse.bass as bass
import concourse.tile as tile
from concourse import bacc, mybir
from concourse.bass_utils import run_bass_kernel_spmd

import os
_HOSTDINV = os.environ.get("HOSTDINV", "0")
_DBG = os.environ.get("DBG", "0") == "1"
_SKIPC = os.environ.get("SKIPC", "0") == "1"
N, F, H = 100000, 512, 16
NCORE = 8
SH = N // NCORE          # 12500 targets per core
WN = 25000               # ap_gather window (nodes per 16-partition group)
NW = N // WN             # 4 windows
ZSLOT = WN               # window-local index of the zero pad column
WPAD = 16                # pad window free dim to 25016 (dummy reads land here)
MAXCALL = 6144           # max slots per ap_gather call (msg tile width)


# ----------------------------------------------------------------------
# host-side layout
# ----------------------------------------------------------------------
def _build_layout(edge_index):
    row = edge_index[0].astype(np.int64)
    col = edge_index[1].astype(np.int64)
    # append self loops
    loop = np.arange(N, dtype=np.int64)
    row = np.concatenate([row, loop])
    col = np.concatenate([col, loop])

    deg = np.bincount(col, minlength=N).astype(np.float64)  # includes self

    # --- balanced source->window assignment (greedy, shard-local caps) ---
    SEG = WN // NCORE                   # 3125 nodes per (shard, window)
    o_src = np.argsort(row, kind="stable")
    rs_, ts_ = row[o_src], col[o_src]
    starts_ = np.searchsorted(rs_, np.arange(N + 1))
    def _greedy(seed):
        rng = np.random.default_rng(seed)
        wofn = np.empty(N, dtype=np.int64)
        segrank = np.empty(N, dtype=np.int64)
        cnt = np.zeros((N, NW), dtype=np.int32)
        for c in range(NCORE):
            nodes = rng.permutation(np.arange(SH * c, SH * (c + 1)))
            cap = np.full(NW, SEG, dtype=np.int64)
            for n in nodes:
                tg = ts_[starts_[n]:starts_[n + 1]]
                costs = cnt[tg].sum(0)
                for wsel in np.argsort(costs, kind="stable"):
                    if cap[wsel] > 0:
                        break
                wofn[n] = wsel
                segrank[n] = SEG - cap[wsel]
                cap[wsel] -= 1
                cnt[tg, wsel] += 1
        ww = wofn[row]
        cgx = np.bincount(col * NW + ww, minlength=N * NW).reshape(N, NW)
        score = np.ceil(cgx / 2).max(1).sum()
        return score, wofn, segrank

    best = min((_greedy(s) for s in (0, 1, 2)), key=lambda r: r[0])
    _, wofn, segrank = best
    woff = SEG * (np.arange(N) // SH) + segrank   # window-local offset
    # shard-local permutation: position SEG*w + r holds the node
    perm = np.empty((NCORE, SH), dtype=np.int64)
    for c in range(NCORE):
        ids = np.arange(SH * c, SH * (c + 1))
        pos = NW * SEG * 0 + wofn[ids] * SEG + segrank[ids]
        p = np.empty(SH, dtype=np.int64)
        p[pos] = ids
        perm[c] = p

    w = wofn[row]                       # source window 0..3
    # occurrence rank within each (col, w) run -> replica + slot
    order = np.lexsort((row, w, col))
    cs, ws, rs = col[order], w[order], row[order]
    key = cs * NW + ws
    newrun = np.ones(len(key), dtype=bool)
    newrun[1:] = key[1:] != key[:-1]
    run_start = np.maximum.accumulate(np.where(newrun, np.arange(len(key)), 0))
    rank = np.arange(len(key)) - run_start
    rep = (rank % 2).astype(np.int64)   # replica 0/1
    slot_k = rank // 2                  # slot within (target, group)
    grp = 4 * rep + ws                  # group 0..7

    # per (col, grp) counts -> per-target cap m(t) = max over groups
    cg = np.bincount(cs * 8 + grp, minlength=N * 8).reshape(N, 8)
    m_t = cg.max(axis=1)                # >= 1 (self loop)
    mmax = int(m_t.max())
    assert mmax <= 128, f"cap {mmax} exceeds msg tile"

    # shared class structure across cores: n_m = max over cores of the
    # number of targets with cap m, padded to a multiple of 16
    counts = np.zeros((NCORE, mmax + 1), dtype=np.int64)
    for c in range(NCORE):
        counts[c] = np.bincount(m_t[SH * c:SH * (c + 1)], minlength=mmax + 1)
    n_m = counts.max(axis=0)
    # multiples of 32 so every sub-call's wrapped idx slice stays 32-bit
    # aligned (the ap_gather ucode reads idxs as int16 pairs)
    n_m = (n_m + 31) // 32 * 32
    # column layout: classes in descending m; pad total columns to 512
    klist = [k for k in range(mmax, 0, -1) if n_m[k] > 0]
    C = int(sum(n_m[k] for k in klist))
    cpad = (-C) % 512
    if cpad:
        if 1 in klist:
            n_m[1] += cpad
        else:
            klist.append(1)
            n_m[1] = cpad
        C += cpad
    col_base = {}
    slot_base = {}
    cb, sb = 0, 0
    for k in klist:
        col_base[k] = cb
        slot_base[k] = sb
        cb += int(n_m[k])
        sb += int(n_m[k]) * k
    NI = sb                              # slots per group
    NI16 = NI // 16

    # sub-call schedule (shared across cores): (col0, n_sub, m, slot_off).
    # Calls never cross a 512-column block so each block's partial tile can
    # be folded + pushed through the MLP as soon as its calls finish.
    calls = []
    for k in klist:
        n_left = int(n_m[k])
        c0 = col_base[k]
        s0 = slot_base[k]
        assert k <= 64
        step = max(32, (MAXCALL // k) // 32 * 32)
        while n_left > 0:
            n_sub = min(n_left, step, 512 - (c0 % 512))
            calls.append((c0, n_sub, k, s0))
            c0 += n_sub
            s0 += n_sub * k
            n_left -= n_sub
    blocks = [[] for _ in range(C // 512)]
    for call in calls:
        blocks[call[0] // 512].append(call)

    # per-core target->column assignment + idx arrays + deg vectors
    idxs_w = np.empty((NCORE, 128, NI16), dtype=np.int16)
    deg_cls = np.ones((NCORE, 1, C), dtype=np.float32)
    col_of_t = np.empty((NCORE, SH), dtype=np.int64)
    next_col = np.empty(mmax + 1, dtype=np.int64)
    for c in range(NCORE):
        for k in klist:
            next_col[k] = col_base[k]
        tl = np.arange(SH * c, SH * (c + 1))
        # stable order: descending m, then target id
        o = np.lexsort((tl, -m_t[tl]))
        ts_sorted = tl[o]
        ms = m_t[ts_sorted]
        # columns assigned in order within each class
        cols = np.empty(SH, dtype=np.int64)
        for k in klist:
            sel = ms == k
            nk = int(sel.sum())
            if nk:
                cols[sel] = next_col[k] + np.arange(nk)
                next_col[k] += nk
        col_of_t[c][ts_sorted - SH * c] = cols
        deg_cls[c, 0, cols] = deg[ts_sorted]

        # slot position of every edge of this core
        emask_s = (cs >= SH * c) & (cs < SH * (c + 1))
        e_t = cs[emask_s]
        e_g = grp[emask_s]
        e_k = slot_k[emask_s]
        e_src = woff[rs[emask_s]]
        e_col = col_of_t[c][e_t - SH * c]
        e_m = m_t[e_t]
        # slot index inside the group's stream
        sbase = np.empty(SH, dtype=np.int64)
        sbase[:] = 0
        for k in klist:
            pass
        sb_of_col = np.empty(C, dtype=np.int64)
        for k in klist:
            c0, s0, nk = col_base[k], slot_base[k], int(n_m[k])
            sb_of_col[c0:c0 + nk] = s0 + np.arange(nk) * k
        pos = sb_of_col[e_col] + e_k
        flat = np.full((8, NI), ZSLOT, dtype=np.int16)
        flat[e_g, pos] = e_src.astype(np.int16)
        # wrap: idx i of group g -> partition 16g + i%16, free i//16
        for g in range(8):
            idxs_w[c, 16 * g:16 * g + 16] = flat[g].reshape(NI16, 16).T

    deg_nat = deg[perm].reshape(NCORE, 1, SH).astype(np.float32)

    sel = np.zeros((128, 16), dtype=np.float32)
    for g in range(8):
        for f in range(16):
            sel[16 * g + f, f] = 1.0

    return dict(C=C, NI=NI, NI16=NI16, calls=calls, blocks=blocks,
                idxs_w=idxs_w,
                deg_cls=deg_cls, deg_nat=deg_nat, col_of_t=col_of_t, sel=sel,
                perm=perm)


# ----------------------------------------------------------------------
# device graph
# ----------------------------------------------------------------------
def _build_graph(C, NI16, blocks):
    f32, bf16, i16 = mybir.dt.float32, mybir.dt.bfloat16, mybir.dt.int16
    nc = bacc.Bacc("TRN2", target_bir_lowering=False, debug=False,
                   num_devices=NCORE)
    xT = nc.dram_tensor("xT", [F, SH], bf16, kind="ExternalInput").ap()
    Wg = nc.dram_tensor("Wg", [F, H], bf16, kind="ExternalInput").ap()
    W1 = nc.dram_tensor("W1", [16, 16], bf16, kind="ExternalInput").ap()
    W2 = nc.dram_tensor("W2", [16, 32], bf16, kind="ExternalInput").ap()
    W3 = nc.dram_tensor("W3", [32, 1], bf16, kind="ExternalInput").ap()
    bg = nc.dram_tensor("bg", [16, 1], f32, kind="ExternalInput").ap()
    b1 = nc.dram_tensor("b1", [16, 1], f32, kind="ExternalInput").ap()
    b2 = nc.dram_tensor("b2", [32, 1], f32, kind="ExternalInput").ap()
    b3 = nc.dram_tensor("b3", [1, 1], f32, kind="ExternalInput").ap()
    selin = nc.dram_tensor("sel", [128, 16], f32, kind="ExternalInput").ap()
    idxin = nc.dram_tensor("idxs", [128, NI16], i16,
                           kind="ExternalInput").ap()
    SHP = 12544                      # SH padded to 128*98
    CP = C // 128
    dnat = nc.dram_tensor("deg_nat", [128, SHP // 128], f32,
                          kind="ExternalInput").ap()
    dcls = nc.dram_tensor("deg_cls", [128, CP], f32,
                          kind="ExternalInput").ap()
    out = nc.dram_tensor("out", [1, C], f32, kind="ExternalOutput").ap()

    dnat_d = nc.dram_tensor("dinv_nat", [128, SHP // 128], f32)
    dcls_d = nc.dram_tensor("dinv_cls", [128, CP], f32)
    SEG = WN // NCORE
    ybw = [nc.dram_tensor(f"ybounce{w}", [H, SEG], f32) for w in range(NW)]
    ygw = [nc.dram_tensor(f"ygather{w}", [NCORE, H, SEG], f32,
                          addr_space="Shared") for w in range(NW)]

    NT = 500                      # node tile (25 tiles per shard)
    with tile.TileContext(nc) as tc:
      with tc.tile_pool(name="ywp", bufs=1) as ywp:
        ywin = ywp.tile([128, WN + WPAD], f32)
        nc.vector.memset(ywin[:, WN:WN + WPAD], 0.0)
        idxs = ywp.tile([128, NI16], i16)
        nc.sync.dma_start(idxs[:], idxin[:])
        # ---------------- phase A: ytil = dinv * (x @ Wg) ----------------
        with tc.tile_pool(name="wgp", bufs=1) as wgp, \
             tc.tile_pool(name="ax", bufs=2) as ax, \
             tc.tile_pool(name="aps", bufs=2, space="PSUM") as aps, \
             tc.tile_pool(name="ay", bufs=3) as ay:
            AFT = mybir.ActivationFunctionType
            for src_ap, dst_ap, wid in ((dnat, dnat_d, SHP // 128),
                                        (dcls, dcls_d, CP)):
                dg = ax.tile([128, wid], f32, tag="dg")
                nc.sync.dma_start(dg[:], src_ap[:, :])
                dgr = ax.tile([128, wid], f32, tag="dgr")
                nc.vector.reciprocal(dgr[:], dg[:])
                dgi = ax.tile([128, wid], f32, tag="dgi")
                nc.scalar.activation(dgi[:], dgr[:], AFT.Sqrt)
                nc.sync.dma_start(dst_ap[:, :], dgi[:])
            dnat_row = dnat_d.ap().rearrange("p j -> () (p j)")
            dcls_row = dcls_d.ap().rearrange("p j -> () (p j)")
            wgt = []
            for kc in range(4):
                t = wgp.tile([128, H], bf16, tag=f"wg{kc}")
                nc.sync.dma_start(t[:], Wg[128 * kc:128 * (kc + 1), :])
                wgt.append(t)
            XB = 2500                    # x DMA batch (5 matmul tiles)
            xbt = [None] * 4
            for nt in range(SH // NT):
                ps = aps.tile([H, NT], f32)
                if nt % (XB // NT) == 0:
                    for kc in range(4):
                        xbt[kc] = ax.tile([128, XB], bf16, name=f"xb{kc}", tag=f"xb{kc}")
                        nc.sync.dma_start(
                            xbt[kc][:],
                            xT[128 * kc:128 * (kc + 1),
                               nt * NT:nt * NT + XB])
                for kc in range(4):
                    nc.tensor.matmul(
                        ps[:], wgt[kc][:],
                        xbt[kc][:, (nt % (XB // NT)) * NT:
                                (nt % (XB // NT) + 1) * NT],
                        start=(kc == 0), stop=(kc == 3))
                di = ay.tile([H, NT], f32, tag="di")
                nc.sync.dma_start(
                    di[:],
                    dnat_row[0:1, bass.ts(nt, NT)].to_broadcast([H, NT]))
                yt = ay.tile([H, NT], f32, tag="yt")
                nc.vector.tensor_tensor(yt[:], ps[:], di[:],
                                        op=mybir.AluOpType.mult)
                # write into the per-segment bounce tensors (may straddle)
                lo = nt * NT
                while lo < (nt + 1) * NT:
                    w = lo // SEG
                    hi = min((nt + 1) * NT, (w + 1) * SEG)
                    nc.sync.dma_start(
                        ybw[w][:, lo - w * SEG:hi - w * SEG],
                        yt[:, lo - nt * NT:hi - nt * NT])
                    lo = hi
                # fire segment-w allgather as soon as segment w is complete
                if ((nt + 1) * NT) // SEG > (nt * NT) // SEG:
                    w = ((nt + 1) * NT) // SEG - 1
                    nc.gpsimd.collective_compute(
                        "AllGather", mybir.AluOpType.bypass,
                        replica_groups=[list(range(NCORE))],
                        ins=[ybw[w].ap().opt()], outs=[ygw[w].ap().opt()])
                    # fill both replica groups of window w off the ACT SEQ
                    for g in (w, w + 4):
                        nc.scalar.dma_start(
                            ywin[16 * g:16 * g + 16, 0:WN].rearrange(
                                "p (cc j) -> p cc j", j=SEG),
                            ygw[w].ap().rearrange("cc f j -> f cc j"))

        # ---------------- phase B: window fill ----------------

        with tc.tile_pool(name="msgp", bufs=2) as msgp, \
             tc.tile_pool(name="pbp", bufs=3) as pbp, \
             tc.tile_pool(name="cst", bufs=1) as cst, \
             tc.tile_pool(name="dps", bufs=2, space="PSUM") as dps, \
             tc.tile_pool(name="dt", bufs=2) as dt:
            selt = cst.tile([128, 16], f32, tag="sel")
            nc.sync.dma_start(selt[:], selin[:])
            w1t = cst.tile([16, 16], bf16, tag="w1")
            nc.sync.dma_start(w1t[:], W1[:])
            w2t = cst.tile([16, 32], bf16, tag="w2")
            nc.sync.dma_start(w2t[:], W2[:])
            w3t = cst.tile([32, 1], bf16, tag="w3")
            nc.sync.dma_start(w3t[:], W3[:])
            bgt = cst.tile([16, 1], f32, tag="bg")
            nc.sync.dma_start(bgt[:], bg[:])
            b1t = cst.tile([16, 1], f32, tag="b1")
            nc.sync.dma_start(b1t[:], b1[:])
            b2t = cst.tile([32, 1], f32, tag="b2")
            nc.sync.dma_start(b2t[:], b2[:])
            b3t = cst.tile([1, 1], f32, tag="b3")
            nc.sync.dma_start(b3t[:], b3[:])

            AFT = mybir.ActivationFunctionType
            AOT = mybir.AluOpType
            for bi, blkcalls in enumerate(blocks):
                sl = bass.ts(bi, 512)
                pb = pbp.tile([128, 512], f32, tag="pb")
                for (c0, n_sub, k, s0) in blkcalls:
                    ns = n_sub * k
                    off = c0 % 512
                    msg = msgp.tile([128, MAXCALL], f32, tag="msg")
                    nc.gpsimd.ap_gather(
                        msg[:, :ns], ywin[:],
                        idxs[:, s0 // 16:(s0 + ns) // 16],
                        channels=128, num_elems=WN + WPAD, d=1, num_idxs=ns)
                    nc.vector.tensor_reduce(
                        pb[:, off:off + n_sub],
                        msg[:, :ns].rearrange("p (n k) -> p n k", k=k),
                        axis=mybir.AxisListType.X, op=AOT.add)
                # fold + epilogue + MLP for this block
                ps = dps.tile([16, 512], f32, tag="ps")
                nc.tensor.matmul(ps[:], selt[:], pb[:, :],
                                 start=True, stop=True)
                dci = dt.tile([16, 512], f32, tag="dci")
                nc.scalar.dma_start(
                    dci[:], dcls_row[0:1, sl].to_broadcast([16, 512]))
                tmp = dt.tile([16, 512], f32, tag="tmp")
                nc.vector.tensor_tensor(tmp[:], ps[:], dci[:], op=AOT.mult)
                h1 = dt.tile([16, 512], bf16, tag="h1")
                nc.vector.tensor_scalar(h1[:], tmp[:], bgt[:, 0:1], 0.0,
                                        op0=AOT.add, op1=AOT.max)
                ps2 = dps.tile([16, 512], f32, tag="ps2")
                nc.tensor.matmul(ps2[:], w1t[:], h1[:], start=True, stop=True)
                h2 = dt.tile([16, 512], bf16, tag="h2")
                nc.vector.tensor_scalar(h2[:], ps2[:], b1t[:, 0:1], 0.0,
                                        op0=AOT.add, op1=AOT.max)
                ps3 = dps.tile([32, 512], f32, tag="ps3")
                nc.tensor.matmul(ps3[:], w2t[:], h2[:], start=True, stop=True)
                h3 = dt.tile([32, 512], bf16, tag="h3")
                nc.vector.tensor_scalar(h3[:], ps3[:], b2t[:, 0:1], 0.0,
                                        op0=AOT.add, op1=AOT.max)
                ps4 = dps.tile([1, 512], f32, tag="ps4")
                nc.tensor.matmul(ps4[:], w3t[:], h3[:], start=True, stop=True)
                ob = dt.tile([1, 512], f32, tag="ob")
                nc.scalar.activation(ob[:], ps4[:], AFT.Sigmoid, bias=b3t[:])
                nc.scalar.dma_start(out[0:1, sl], ob[:])
    nc.compile()
    return nc


# ----------------------------------------------------------------------
# entry point
# ----------------------------------------------------------------------
def kernel(x, edge_index, Wg, bg, W1, b1, W2, b2, W3, b3):
    lay = _build_layout(np.asarray(edge_index))
    nc = _build_graph(lay["C"], lay["NI16"], lay["blocks"])

    import ml_dtypes
    bf16np = ml_dtypes.bfloat16
    shared = {
        "Wg": np.ascontiguousarray(np.asarray(Wg, np.float32)).astype(bf16np),
        "W1": np.ascontiguousarray(np.asarray(W1, np.float32)).astype(bf16np),
        "W2": np.ascontiguousarray(np.asarray(W2, np.float32)).astype(bf16np),
        "W3": np.ascontiguousarray(np.asarray(W3, np.float32)).astype(bf16np),
        "bg": np.asarray(bg, np.float32).reshape(16, 1),
        "b1": np.asarray(b1, np.float32).reshape(16, 1),
        "b2": np.asarray(b2, np.float32).reshape(32, 1),
        "b3": np.asarray(b3, np.float32).reshape(1, 1),
        "sel": lay["sel"],
    }
    x = np.asarray(x, np.float32)
    in_maps = []
    for c in range(NCORE):
        m = dict(shared)
        m["xT"] = np.ascontiguousarray(x[lay["perm"][c]].T).astype(bf16np)
        m["idxs"] = lay["idxs_w"][c]
        dn = np.ones(12544, np.float32)
        dn[:SH] = lay["deg_nat"][c].reshape(-1)
        m["deg_nat"] = dn.reshape(128, 98)
        m["deg_cls"] = np.ascontiguousarray(
            lay["deg_cls"][c].reshape(-1).reshape(128, -1))
        in_maps.append(m)

    res = run_bass_kernel_spmd(nc, in_maps, core_ids=list(range(NCORE)))
    out = np.empty((N, 1), np.float32)
    for c in range(NCORE):
        oc = res.results[c]["out"].reshape(-1)
        out[SH * c:SH * (c + 1), 0] = oc[lay["col_of_t"][c]]
    return out
